# revision 44
# baseline (speedup 1.0000x reference)
"""Pointer-style attention kernel for Trainium2, SPMD over 8 NeuronCores.

Reference computation (per full batch B=128, S=2048, E=H=512):
    q  = query @ Wq.T + bq                    [B, H]
    k  = target @ Wk.T + bk                   [B, S, H]
    qk = einsum('bh,bsh->bs', q, k)           [B, S]
    qk = 10 * tanh(qk);  qk[mask==1] = -inf
    alpha = softmax(qk, axis=-1)

Key algebraic reformulation (exact in exact arithmetic):
    qk[b,s] = target[b,s,:] . qp[b,:] + qb[b]
      qp = (query @ Wq.T + bq) @ Wk           [B, E]
      qb = query @ (Wq.T @ bk) + bq . bk      [B]
This collapses the S*E*H einsum (137 GFLOP) into an S*E dot-product
stream, making the kernel HBM-bound on streaming `target`.

Masked-row skip: entries with mask==1 get alpha==0 exactly (softmax of
-inf), so only the ~half of target rows with mask==0 are ever needed.
The host packs those rows contiguously per batch (a pure data-layout
restage, like the existing weight transposes), the device computes the
dense masked softmax over the packed rows, and the host scatters the
gathered alpha back into the full [B, S] output with zeros elsewhere.
This halves both the HBM stream and the VectorE work.

Distribution: data-parallel over batch; 16 batches per core, weights
replicated, no cross-core communication (softmax is per-row).

Per-core plan:
  - critical weights (queryT/wqT/wk) ride the head of the sync queue;
    TensorE builds q -> qhT(+bq) -> qp = qhT^T @ Wk ONCE ([BS, E]), and
    per-batch [128, E] broadcasts cost a single 16-row ident-select
    matmul each (PE stays nearly idle = less power throttling)
  - main loop streams packed target ([128, KG, 512] full-batch units)
    via HWDGE DMA; VectorE fused scalar_tensor_tensor ops compute
    scores = sum_e target[s,e]*qp[e] per 128-row chunk in one pass
    (accum_out; the product goes to a stride-0 dummy)
  - epilogue: +qb, tanh/exp on ScalarE, validity masking (host-built
    m01g), per-row softmax in an s-on-partitions layout (cross-partition
    sums via ones matmuls), TensorE transpose, contiguous DMA out.
"""

import sys
import types

import numpy as np

B, S, E, H = 128, 2048, 512, 512
C_CLIP = 10.0
NCORES = 8
BS = B // NCORES  # 16 batches per core
EC = E // 128  # 4 e-chunks of 128


def _install_axon_profile_shim():
    """Make run_bass_kernel_spmd(trace=True) usable in this container:
    provide antenv.axon_hooks (NTFF profile hook via ctypes into the
    axon PJRT .so) and stub the S3 artifact upload."""
    try:
        if "antenv.axon_hooks" not in sys.modules:
            import antenv
            from trn_agent_boot.trn_boot import _ntff_profile_via_ctypes

            hook = _ntff_profile_via_ctypes("/opt/axon/libaxon_pjrt.so")
            mod = types.ModuleType("antenv.axon_hooks")
            mod._hook = hook
            mod.get_axon_ntff_profile_hook = lambda: mod._hook

            def _set(h):
                mod._hook = h

            mod.set_axon_ntff_profile_hook = _set
            sys.modules["antenv.axon_hooks"] = mod
            antenv.axon_hooks = mod
    except Exception:
        pass
    try:
        import concourse.bass_utils as bu

        bu.upload_artifacts = lambda tmpdir: str(tmpdir)
    except Exception:
        pass


def _legalize_sync_waits(nc):
    """This walrus build rejects instructions carrying more than a couple
    of sync-wait commands. After Tile scheduling, split each instruction's
    excess waits onto same-engine NOPs inserted immediately before it --
    sequencers execute in order, so semantics are identical."""
    import bass_rust
    from concourse import mybir

    n_split = 0
    for f in nc.m.functions:
        for blk in f.blocks:
            il = blk.instructions
            out = []
            changed = False
            for inst in il:
                si = inst.sync_info
                waits = list(si.on_wait) if si is not None else []
                cap = 2 if isinstance(inst, mybir.InstEventSemaphore) else 1
                if len(waits) > cap:
                    rest = waits[: len(waits) - cap]
                    for j, w in enumerate(rest):
                        nop = mybir.InstNoOp(
                            name=f"{inst.name}-swait{j}",
                            engine=inst.engine,
                            bass_nofuse=True,
                            sync_info=bass_rust.SyncInfo(on_wait=[w], on_update=[]),
                        )
                        out.append(nop)
                        n_split += 1
                    si.on_wait = waits[len(waits) - cap :]
                    inst.sync_info = si
                    changed = True
                out.append(inst)
            if changed:
                blk.instructions = out
    return n_split


def build_kernel(nidx, skip_qb=False):
    """skip_qb: when the host verifies bk == 0, qb = q.bk + bq.bk is
    identically zero, so the whole side chain (and the Wq stream) is
    elided and the epilogue tanh reads scores directly."""
    import concourse.bass as bass
    import concourse.tile as tile
    from concourse import mybir
    from concourse.masks import make_identity

    f32 = mybir.dt.float32
    Alu = mybir.AluOpType
    Act = mybir.ActivationFunctionType

    KG = nidx // 128  # packed s-chunks per batch

    nc = bass.Bass()
    # host passes layout-transformed views: queryT/WqT transposed, bq in
    # [p, chunk] column form, target packed to unmasked rows, m01g the
    # packed-row validity mask (1.0 real / 0.0 padding)
    queryT_d = nc.dram_tensor("queryT", [E, BS], f32, kind="ExternalInput")
    target_d = nc.dram_tensor("target", [BS, nidx, E], f32, kind="ExternalInput")
    m01g_d = nc.dram_tensor("m01g", [128, BS, KG], f32, kind="ExternalInput")
    wq_d = nc.dram_tensor("Wq", [H, E], f32, kind="ExternalInput")
    wqT_d = nc.dram_tensor("WqT", [E, H], f32, kind="ExternalInput")
    bqT_d = nc.dram_tensor("bqT", [128, EC], f32, kind="ExternalInput")
    wk_d = nc.dram_tensor("Wk", [H, E], f32, kind="ExternalInput")
    bkT_d = nc.dram_tensor("bkT", [128, EC], f32, kind="ExternalInput")
    alpha_d = nc.dram_tensor("alpha", [BS, nidx], f32, kind="ExternalOutput")

    n_units = BS  # 16 full-batch pipeline units of [128, KG, E]

    with tile.TileContext(nc) as tc:
        with (
            tc.tile_pool(name="singles", bufs=1) as singles,
            tc.tile_pool(name="tgt", bufs=7) as tgtp,
            tc.tile_pool(name="trash", bufs=2) as trashp,
            tc.tile_pool(name="pbs", bufs=3) as pbsp,
            tc.tile_pool(name="ppre", bufs=2, space="PSUM") as ppre,
            tc.tile_pool(name="pqpb", bufs=3, space="PSUM") as pqpb,
            tc.tile_pool(name="pepi", bufs=2, space="PSUM") as pepi,
        ):
            # ================= critical-path preamble =================
            # The q -> qhT -> qp chain gates the first VectorE op, so its
            # DMAs go at the HEAD of the sync queue, in front of the
            # target stream (the stream finishes well before VectorE
            # anyway). Non-critical inputs ride the ScalarE queue.
            qT_sb = singles.tile([128, EC, BS], f32)  # queryT [e'-part, b]
            nc.sync.dma_start(out=qT_sb, in_=queryT_d.rearrange("(m p) b -> p m b", p=128))
            wqT_sb = singles.tile([128, EC, H], f32)  # [p, e'-chunk, h]
            wk_sb = singles.tile([128, EC, E], f32)   # [p, h-chunk, e]
            for c in range(EC):
                nc.sync.dma_start(out=wqT_sb[:, c, :], in_=wqT_d[c * 128 : (c + 1) * 128, :])
            # wk is needed ~5us later (after q/qhT), so it rides the ScalarE
            # queue in parallel with the target stream
            for c in range(EC):
                nc.scalar.dma_start(out=wk_sb[:, c, :], in_=wk_d[c * 128 : (c + 1) * 128, :])
            bqT = singles.tile([128, EC], f32)
            bkT = singles.tile([128, EC], f32)
            nc.scalar.dma_start(out=bqT, in_=bqT_d[:, :])
            nc.scalar.dma_start(out=bkT, in_=bkT_d[:, :])
            m01g_sb = singles.tile([128, BS, KG], f32)
            nc.scalar.dma_start(out=m01g_sb, in_=m01g_d[:, :, :])

            ident = singles.tile([128, 128], f32)
            make_identity(nc, ident)
            ones_row = singles.tile([1, 128], f32)  # lhsT for partition-bcast
            nc.vector.memset(ones_row, 1.0)
            ones_col = singles.tile([128, 1], f32)  # lhsT/rhs for partition-sum
            nc.vector.memset(ones_col, 1.0)

            # target stream: issue every unit's DMA on the sync queue; the
            # tile pool (bufs) backpressures the stream automatically. The
            # first units' DMAs are hoisted here, ahead of everything else
            # emitted on the sync HWDGE queue, so tgt0 lands right behind
            # the wqT chunks instead of queuing behind later ring slots.
            target_units = target_d.rearrange("b (k p) e -> b p k e", p=128)

            # Warm the PE clock gate (HAM) during the weight DMAs so the
            # q -> qhT -> qp chain runs at full clock.
            for i in range(26):
                pwrm = ppre.tile([128, 128], f32, tag="pre")
                nc.tensor.matmul(pwrm, ident, ident, start=True, stop=True)

            # q = query @ Wq.T  [BS, H]
            q_sb = singles.tile([BS, H], f32)
            pq2 = ppre.tile([BS, H], f32, tag="pre")
            for m in range(EC):
                nc.tensor.matmul(pq2, qT_sb[:, m, :], wqT_sb[:, m, :],
                                 start=(m == 0), stop=(m == EC - 1))
            nc.scalar.copy(q_sb, pq2)
            # qhT = (q + bq) transposed to [h-part, b]; copy+bias on ScalarE
            # so VectorE stays free for the target stream
            qhT_sb = singles.tile([128, EC, BS], f32)
            for c in range(EC):
                pq3 = ppre.tile([128, BS], f32, tag="pre")
                nc.tensor.transpose(pq3, q_sb[:, c * 128 : (c + 1) * 128], ident[0:BS, 0:BS])
                nc.scalar.activation(qhT_sb[:, c, :], pq3, Act.Identity,
                                     bias=bqT[:, c : c + 1], scale=1.0)

            # qp = (q + bq) @ Wk  [BS, E], computed ONCE: 4 accumulating
            # matmuls instead of per-batch 128-row broadcast matmuls. The
            # per-batch [128, E] broadcast then costs a single 16-row
            # ident-select matmul, slashing PE busy time (and power).
            qp_sb = singles.tile([BS, E], f32)
            pqp = ppre.tile([BS, E], f32, tag="pre")
            for c in range(EC):
                nc.tensor.matmul(pqp, qhT_sb[:, c, :], wk_sb[:, c, :],
                                 start=(c == 0), stop=(c == EC - 1))
            nc.scalar.copy(qp_sb, pqp)

            # ============ side chain (emitted at u==2) ================
            # qb[b] = q . bk + bq . bk, broadcast to [128, BS]. Only
            # consumed by the epilogue. (Mask processing is gone: the
            # host-built m01g already has the packed validity layout.)
            def _side_chain():
                # v[e'] = Wq.T @ bk, qb_raw = query @ v
                wq_sb = singles.tile([128, EC, E], f32)  # natural [p, h-chunk, e']
                for c in range(EC):
                    nc.scalar.dma_start(out=wq_sb[:, c, :], in_=wq_d[c * 128 : (c + 1) * 128, :])
                v_sb = singles.tile([128, EC], f32)
                for m in range(EC):
                    pv = ppre.tile([128, 1], f32, tag="pre")
                    for c in range(EC):
                        nc.tensor.matmul(pv, wq_sb[:, c, m * 128 : (m + 1) * 128],
                                         bkT[:, c : c + 1],
                                         start=(c == 0), stop=(c == EC - 1))
                    nc.vector.tensor_copy(v_sb[:, m : m + 1], pv)
                qb_sb = singles.tile([BS, 1], f32)
                pqbv = ppre.tile([BS, 1], f32, tag="pre")
                for m in range(EC):
                    nc.tensor.matmul(pqbv, qT_sb[:, m, :], v_sb[:, m : m + 1],
                                     start=(m == 0), stop=(m == EC - 1))
                nc.vector.tensor_copy(qb_sb, pqbv)
                # dot(bq, bk)
                trash4 = singles.tile([128, EC], f32)
                dotp = singles.tile([128, 1], f32)
                nc.vector.tensor_mul(trash4, bqT, bkT)
                nc.vector.tensor_reduce(dotp, trash4, axis=mybir.AxisListType.X, op=Alu.add)
                pdot = ppre.tile([1, 1], f32, tag="pre")
                nc.tensor.matmul(pdot, dotp, ones_col, start=True, stop=True)
                dot_sb = singles.tile([1, 1], f32)
                nc.vector.tensor_copy(dot_sb, pdot)
                pqbrow = ppre.tile([1, BS], f32, tag="pre")
                nc.tensor.transpose(pqbrow, qb_sb, ident[0:BS, 0:BS])
                qbrow_sb = singles.tile([1, BS], f32)
                nc.scalar.activation(qbrow_sb, pqbrow, Act.Identity,
                                     bias=dot_sb[0:1, 0:1], scale=1.0)
                pqbb = ppre.tile([128, BS], f32, tag="pre")
                nc.tensor.matmul(pqbb, ones_row, qbrow_sb, start=True, stop=True)
                qbb = singles.tile([128, BS], f32)
                nc.vector.tensor_copy(qbb, pqbb)
                st["qbb"] = qbb

            # ============ epilogue (two halves of 8 batches) ==========
            e2_sb = singles.tile([128, BS, KG], f32)
            a_sb = singles.tile([128, BS, KG], f32)
            part_sb = singles.tile([128, BS], f32)
            alpha_flat = alpha_d.rearrange("b (k p) -> (b k) p", p=128)
            HW = (BS // 2) * KG  # transpose width per half

            def _epi_half(half):
                b0, b1 = half * (BS // 2), (half + 1) * (BS // 2)
                eng_ts = nc.vector
                if skip_qb:
                    # one cheap DVE copy: scores reads stay confined to the
                    # VectorE program order, so later units' accum writes
                    # don't pick up cross-engine WAR stalls against the
                    # ScalarE tanh below
                    scores2 = singles.tile([128, BS // 2, KG], f32, tag=f"sc2_{half}")
                    nc.vector.tensor_copy(scores2, scores[:, b0:b1, :])
                    tanh_in = scores2
                else:
                    qbb = st["qbb"]
                    scores2 = singles.tile([128, BS // 2, KG], f32, tag=f"sc2_{half}")
                    for b in range(b0, b1):
                        eng_ts.tensor_scalar(
                            out=scores2[:, b - b0, :], in0=scores[:, b, :],
                            scalar1=qbb[:, b : b + 1], scalar2=None, op0=Alu.add,
                        )
                    tanh_in = scores2
                t_sb = singles.tile([128, BS // 2, KG], f32, tag=f"t_{half}")
                nc.scalar.activation(t_sb, tanh_in, Act.Tanh)
                nc.scalar.activation(e2_sb[:, b0:b1, :], t_sb, Act.Exp, scale=C_CLIP)
                nc.vector.tensor_mul(e2_sb[:, b0:b1, :], e2_sb[:, b0:b1, :],
                                     m01g_sb[:, b0:b1, :])
                # one 3D reduce over the innermost (chunk) dim replaces 8
                # serial per-batch reduces on the critical tail
                nc.vector.tensor_reduce(
                    part_sb[:, b0:b1], e2_sb[:, b0:b1, :],
                    axis=mybir.AxisListType.X, op=Alu.add,
                )
                pden = pepi.tile([1, BS // 2], f32, tag="epi")
                nc.tensor.matmul(pden, ones_col, part_sb[:, b0:b1], start=True, stop=True)
                recip_sb = singles.tile([1, BS // 2], f32, tag=f"rc_{half}")
                nc.vector.reciprocal(recip_sb, pden)
                prb = pepi.tile([128, BS // 2], f32, tag="epi")
                nc.tensor.matmul(prb, ones_row, recip_sb, start=True, stop=True)
                rb_sb = singles.tile([128, BS // 2], f32, tag=f"rb_{half}")
                nc.scalar.copy(rb_sb, prb)
                # one tensor_tensor with rb broadcast along the chunk dim
                # (stride-0 AP) replaces 8 serial per-batch muls
                rbb = bass.AP(
                    tensor=rb_sb.tensor, offset=rb_sb.offset,
                    ap=[rb_sb.ap[0], rb_sb.ap[1], [0, KG]],
                )
                eng_ts.tensor_mul(a_sb[:, b0:b1, :], e2_sb[:, b0:b1, :], rbb)
                pat = pepi.tile([128, 128], f32, tag="epi")
                nc.tensor.transpose(pat[0:HW, 0:128], a_sb[:, b0:b1, :], ident)
                at_sb = singles.tile([128, 128], f32, tag=f"at_{half}")
                nc.scalar.copy(at_sb[0:HW, 0:128], pat[0:HW, 0:128])
                nc.sync.dma_start(out=alpha_flat[half * HW : (half + 1) * HW, :],
                                  in_=at_sb[0:HW, 0:128])

            st = {}
            _emit_side_chain = _side_chain
            _epilogue_half = _epi_half

            # ================= main pipeline ==========================
            # Full-batch units: KG-chunk DMA -> KG fused
            # scalar_tensor_tensor ops on VectorE, each multiplying a
            # [128, 512] packed s-chunk by the SBUF-resident qp broadcast
            # with accum_out reducing over e in the same pass. out is a
            # stride-0 dummy column (the product is never stored).
            scores = singles.tile([128, BS, KG], f32)
            for u in range(n_units):
                b = u
                # broadcast qp[b] across 128 partitions with a single
                # 16-row matmul: lhsT = ident column b replicated along
                # the free dim (stride 0) selects row b of qp_sb into
                # every output partition
                sel = bass.AP(
                    tensor=ident.tensor,
                    offset=ident[0:BS, b : b + 1].offset,
                    ap=[[ident.ap[0][0], BS], [0, 128]],
                )
                pb_ps = pqpb.tile([128, E], f32, tag="qpb")
                nc.tensor.matmul(pb_ps, sel, qp_sb[:, :], start=True, stop=True)
                # stage the broadcast in SBUF: DVE pays 58-cycle access
                # latency per op instead of PSUM's 120
                pb_cur = pbsp.tile([128, E], f32, tag="pbs")
                nc.scalar.copy(pb_cur, pb_ps)
                tgt = tgtp.tile([128, KG, E], f32, tag="tgt")
                nc.sync.dma_start(out=tgt, in_=target_units[b])
                for k in range(KG):
                    tr = trashp.tile([128, 1], f32, tag="trash")
                    nc.vector.scalar_tensor_tensor(
                        out=tr.broadcast_to(tgt[:, k, :].shape),
                        in0=tgt[:, k, :], scalar=1.0, in1=pb_cur,
                        op0=Alu.mult, op1=Alu.mult,
                        accum_out=scores[:, b, k : k + 1],
                    )
                if u == 2 and not skip_qb:
                    _emit_side_chain()
                if u == 11:
                    _epilogue_half(0)

            _epilogue_half(1)

    _legalize_sync_waits(nc)
    return nc


_NC_CACHE = None
_NC_KEY = None


def _pack_inputs(query, target, mask, Wq, bq, Wk, bk):
    """Host-side restage: per-row unmasked indices, packed target rows,
    validity mask in the device's [p, b, k] layout."""
    idxs = [np.flatnonzero(mask[r] == 0) for r in range(B)]
    nmax = max((len(ix) for ix in idxs), default=1)
    nidx = max(128, ((nmax + 127) // 128) * 128)
    kg = nidx // 128
    packed = np.zeros((B, nidx, E), dtype=np.float32)
    m01g = np.zeros((128, B, kg), dtype=np.float32)
    for r in range(B):
        n = len(idxs[r])
        packed[r, :n] = target[r, idxs[r]]
        flat = np.zeros(nidx, dtype=np.float32)
        flat[:n] = 1.0
        m01g[:, r, :] = flat.reshape(kg, 128).T
    return idxs, nidx, packed, m01g


def kernel(query, target, mask, Wq, bq, Wk, bk):
    global _NC_CACHE, _NC_KEY
    _install_axon_profile_shim()
    from concourse.bass_utils import run_bass_kernel_spmd

    query = np.ascontiguousarray(np.asarray(query, dtype=np.float32))
    target = np.ascontiguousarray(np.asarray(target, dtype=np.float32))
    mask = np.ascontiguousarray(np.asarray(mask, dtype=np.int32))
    Wq = np.ascontiguousarray(np.asarray(Wq, dtype=np.float32))
    bq = np.ascontiguousarray(np.asarray(bq, dtype=np.float32))
    Wk = np.ascontiguousarray(np.asarray(Wk, dtype=np.float32))
    bk = np.ascontiguousarray(np.asarray(bk, dtype=np.float32))

    in_maps, idxs, nidx = _make_in_maps_impl(query, target, mask, Wq, bq, Wk, bk)

    skip_qb = bool(np.all(bk == 0.0))
    if _NC_CACHE is None or _NC_KEY != (nidx, skip_qb):
        _NC_CACHE = build_kernel(nidx, skip_qb)
        _NC_KEY = (nidx, skip_qb)
    nc = _NC_CACHE

    res = run_bass_kernel_spmd(nc, in_maps, list(range(NCORES)))
    out = np.zeros((B, S), dtype=np.float32)
    for i in range(NCORES):
        ag = res.results[i]["alpha"]
        for b in range(BS):
            r = i * BS + b
            n = len(idxs[r])
            out[r, idxs[r]] = ag[b, :n]
    return out


def _make_in_maps_impl(query, target, mask, Wq, bq, Wk, bk):
    idxs, nidx, packed, m01g = _pack_inputs(query, target, mask, Wq, bq, Wk, bk)
    WqT = np.ascontiguousarray(Wq.T)
    bqT = np.ascontiguousarray(bq.reshape(EC, 128).T)
    bkT = np.ascontiguousarray(bk.reshape(EC, 128).T)
    in_maps = []
    for i in range(NCORES):
        sl = slice(i * BS, (i + 1) * BS)
        in_maps.append(
            {
                "queryT": np.ascontiguousarray(query[sl].T),
                "target": np.ascontiguousarray(packed[sl]),
                "m01g": np.ascontiguousarray(m01g[:, sl, :]),
                "Wq": Wq,
                "WqT": WqT,
                "bqT": bqT,
                "Wk": Wk,
                "bkT": bkT,
            }
        )
    return in_maps, idxs, nidx


def make_in_maps(query, target, mask, Wq, bq, Wk, bk):
    in_maps, _, _ = _make_in_maps_impl(
        np.asarray(query, dtype=np.float32),
        np.asarray(target, dtype=np.float32),
        np.asarray(mask, dtype=np.int32),
        np.asarray(Wq, dtype=np.float32),
        np.asarray(bq, dtype=np.float32),
        np.asarray(Wk, dtype=np.float32),
        np.asarray(bk, dtype=np.float32),
    )
    return in_maps


# revision 48
# speedup vs baseline: 1.0012x; 1.0012x over previous
"""Pointer-style attention kernel for Trainium2, SPMD over 8 NeuronCores.

Reference computation (per full batch B=128, S=2048, E=H=512):
    q  = query @ Wq.T + bq                    [B, H]
    k  = target @ Wk.T + bk                   [B, S, H]
    qk = einsum('bh,bsh->bs', q, k)           [B, S]
    qk = 10 * tanh(qk);  qk[mask==1] = -inf
    alpha = softmax(qk, axis=-1)

Key algebraic reformulation (exact in exact arithmetic):
    qk[b,s] = target[b,s,:] . qp[b,:] + qb[b]
      qp = (query @ Wq.T + bq) @ Wk           [B, E]
      qb = query @ (Wq.T @ bk) + bq . bk      [B]
This collapses the S*E*H einsum (137 GFLOP) into an S*E dot-product
stream, making the kernel HBM-bound on streaming `target`.

Masked-row skip: entries with mask==1 get alpha==0 exactly (softmax of
-inf), so only the ~half of target rows with mask==0 are ever needed.
The host packs those rows contiguously per batch (a pure data-layout
restage, like the existing weight transposes), the device computes the
dense masked softmax over the packed rows, and the host scatters the
gathered alpha back into the full [B, S] output with zeros elsewhere.
This halves both the HBM stream and the VectorE work.

Distribution: data-parallel over batch; 16 batches per core, weights
replicated, no cross-core communication (softmax is per-row).

Per-core plan:
  - critical weights (queryT/wqT/wk) ride the head of the sync queue;
    TensorE builds q -> qhT(+bq) -> qp = qhT^T @ Wk ONCE ([BS, E]), and
    per-batch [128, E] broadcasts cost a single 16-row ident-select
    matmul each (PE stays nearly idle = less power throttling)
  - main loop streams packed target ([128, KG, 512] full-batch units)
    via HWDGE DMA; VectorE fused scalar_tensor_tensor ops compute
    scores = sum_e target[s,e]*qp[e] per 128-row chunk in one pass
    (accum_out; the product goes to a stride-0 dummy)
  - epilogue: +qb, tanh/exp on ScalarE, validity masking (host-built
    m01g), per-row softmax in an s-on-partitions layout (cross-partition
    sums via ones matmuls), TensorE transpose, contiguous DMA out.
"""

import sys
import types

import numpy as np

B, S, E, H = 128, 2048, 512, 512
C_CLIP = 10.0
NCORES = 8
BS = B // NCORES  # 16 batches per core
EC = E // 128  # 4 e-chunks of 128


def _install_axon_profile_shim():
    """Make run_bass_kernel_spmd(trace=True) usable in this container:
    provide antenv.axon_hooks (NTFF profile hook via ctypes into the
    axon PJRT .so) and stub the S3 artifact upload."""
    try:
        if "antenv.axon_hooks" not in sys.modules:
            import antenv
            from trn_agent_boot.trn_boot import _ntff_profile_via_ctypes

            hook = _ntff_profile_via_ctypes("/opt/axon/libaxon_pjrt.so")
            mod = types.ModuleType("antenv.axon_hooks")
            mod._hook = hook
            mod.get_axon_ntff_profile_hook = lambda: mod._hook

            def _set(h):
                mod._hook = h

            mod.set_axon_ntff_profile_hook = _set
            sys.modules["antenv.axon_hooks"] = mod
            antenv.axon_hooks = mod
    except Exception:
        pass
    try:
        import concourse.bass_utils as bu

        bu.upload_artifacts = lambda tmpdir: str(tmpdir)
    except Exception:
        pass


def _legalize_sync_waits(nc):
    """This walrus build rejects instructions carrying more than a couple
    of sync-wait commands. After Tile scheduling, split each instruction's
    excess waits onto same-engine NOPs inserted immediately before it --
    sequencers execute in order, so semantics are identical."""
    import bass_rust
    from concourse import mybir

    n_split = 0
    for f in nc.m.functions:
        for blk in f.blocks:
            il = blk.instructions
            out = []
            changed = False
            for inst in il:
                si = inst.sync_info
                waits = list(si.on_wait) if si is not None else []
                cap = 2 if isinstance(inst, mybir.InstEventSemaphore) else 1
                if len(waits) > cap:
                    rest = waits[: len(waits) - cap]
                    for j, w in enumerate(rest):
                        nop = mybir.InstNoOp(
                            name=f"{inst.name}-swait{j}",
                            engine=inst.engine,
                            bass_nofuse=True,
                            sync_info=bass_rust.SyncInfo(on_wait=[w], on_update=[]),
                        )
                        out.append(nop)
                        n_split += 1
                    si.on_wait = waits[len(waits) - cap :]
                    inst.sync_info = si
                    changed = True
                out.append(inst)
            if changed:
                blk.instructions = out
    return n_split


def build_kernel(nidx, skip_qb=False):
    """skip_qb: when the host verifies bk == 0, qb = q.bk + bq.bk is
    identically zero, so the whole side chain (and the Wq stream) is
    elided and the epilogue tanh reads scores directly."""
    import concourse.bass as bass
    import concourse.tile as tile
    from concourse import mybir
    from concourse.masks import make_identity

    f32 = mybir.dt.float32
    Alu = mybir.AluOpType
    Act = mybir.ActivationFunctionType

    KG = nidx // 128  # packed s-chunks per batch

    nc = bass.Bass()
    # host passes layout-transformed views: queryT/WqT transposed, bq in
    # [p, chunk] column form, target packed to unmasked rows, m01g the
    # packed-row validity mask (1.0 real / 0.0 padding)
    queryT_d = nc.dram_tensor("queryT", [E, BS], f32, kind="ExternalInput")
    target_d = nc.dram_tensor("target", [BS, nidx, E], f32, kind="ExternalInput")
    m01g_d = nc.dram_tensor("m01g", [128, BS, KG], f32, kind="ExternalInput")
    wq_d = nc.dram_tensor("Wq", [H, E], f32, kind="ExternalInput")
    wqT_d = nc.dram_tensor("WqT", [E, H], f32, kind="ExternalInput")
    bqT_d = nc.dram_tensor("bqT", [128, EC], f32, kind="ExternalInput")
    wk_d = nc.dram_tensor("Wk", [H, E], f32, kind="ExternalInput")
    bkT_d = nc.dram_tensor("bkT", [128, EC], f32, kind="ExternalInput")
    alpha_d = nc.dram_tensor("alpha", [BS, nidx], f32, kind="ExternalOutput")

    n_units = BS  # 16 full-batch pipeline units of [128, KG, E]

    with tile.TileContext(nc) as tc:
        with (
            tc.tile_pool(name="singles", bufs=1) as singles,
            tc.tile_pool(name="tgt", bufs=7) as tgtp,
            tc.tile_pool(name="trash", bufs=2) as trashp,
            tc.tile_pool(name="pbs", bufs=3) as pbsp,
            tc.tile_pool(name="ppre", bufs=2, space="PSUM") as ppre,
            tc.tile_pool(name="pqpb", bufs=3, space="PSUM") as pqpb,
            tc.tile_pool(name="pepi", bufs=2, space="PSUM") as pepi,
        ):
            # ================= critical-path preamble =================
            # The q -> qhT -> qp chain gates the first VectorE op, so its
            # DMAs go at the HEAD of the sync queue, in front of the
            # target stream (the stream finishes well before VectorE
            # anyway). Non-critical inputs ride the ScalarE queue.
            qT_sb = singles.tile([128, EC, BS], f32)  # queryT [e'-part, b]
            nc.sync.dma_start(out=qT_sb, in_=queryT_d.rearrange("(m p) b -> p m b", p=128))
            wqT_sb = singles.tile([128, EC, H], f32)  # [p, e'-chunk, h]
            wk_sb = singles.tile([128, EC, E], f32)   # [p, h-chunk, e]
            for c in range(EC):
                nc.sync.dma_start(out=wqT_sb[:, c, :], in_=wqT_d[c * 128 : (c + 1) * 128, :])
            # wk is needed ~5us later (after q/qhT), so it rides the ScalarE
            # queue in parallel with the target stream
            for c in range(EC):
                nc.scalar.dma_start(out=wk_sb[:, c, :], in_=wk_d[c * 128 : (c + 1) * 128, :])
            bqT = singles.tile([128, EC], f32)
            bkT = singles.tile([128, EC], f32)
            nc.scalar.dma_start(out=bqT, in_=bqT_d[:, :])
            nc.scalar.dma_start(out=bkT, in_=bkT_d[:, :])
            m01g_sb = singles.tile([128, BS, KG], f32)
            nc.scalar.dma_start(out=m01g_sb, in_=m01g_d[:, :, :])

            ident = singles.tile([128, 128], f32)
            make_identity(nc, ident)
            ones_row = singles.tile([1, 128], f32)  # lhsT for partition-bcast
            nc.vector.memset(ones_row, 1.0)
            ones_col = singles.tile([128, 1], f32)  # lhsT/rhs for partition-sum
            nc.vector.memset(ones_col, 1.0)

            # target stream: issue every unit's DMA on the sync queue; the
            # tile pool (bufs) backpressures the stream automatically. The
            # first units' DMAs are hoisted here, ahead of everything else
            # emitted on the sync HWDGE queue, so tgt0 lands right behind
            # the wqT chunks instead of queuing behind later ring slots.
            target_units = target_d.rearrange("b (k p) e -> b p k e", p=128)

            # Warm the PE clock gate (HAM) during the weight DMAs so the
            # q -> qhT -> qp chain runs at full clock.
            for i in range(26):
                pwrm = ppre.tile([128, 128], f32, tag="pre")
                nc.tensor.matmul(pwrm, ident, ident, start=True, stop=True)

            # qhT[h, b] = (query @ Wq.T)[b, h] computed DIRECTLY as
            # wqT_c^T @ qT_c accumulating chunks -- no q intermediate, no
            # PSUM round trip, no transposes on the critical chain. The
            # +bq bias rides the PSUM->SBUF copy on ScalarE.
            qhT_sb = singles.tile([128, EC, BS], f32)
            for hc in range(EC):
                pq3 = ppre.tile([128, BS], f32, tag="pre")
                for c in range(EC):
                    nc.tensor.matmul(pq3, wqT_sb[:, c, hc * 128 : (hc + 1) * 128],
                                     qT_sb[:, c, :],
                                     start=(c == 0), stop=(c == EC - 1))
                nc.scalar.activation(qhT_sb[:, hc, :], pq3, Act.Identity,
                                     bias=bqT[:, hc : hc + 1], scale=1.0)

            # qp = (q + bq) @ Wk  [BS, E], computed ONCE: 4 accumulating
            # matmuls instead of per-batch 128-row broadcast matmuls. The
            # per-batch [128, E] broadcast then costs a single 16-row
            # ident-select matmul, slashing PE busy time (and power).
            qp_sb = singles.tile([BS, E], f32)
            pqp = ppre.tile([BS, E], f32, tag="pre")
            for c in range(EC):
                nc.tensor.matmul(pqp, qhT_sb[:, c, :], wk_sb[:, c, :],
                                 start=(c == 0), stop=(c == EC - 1))
            nc.scalar.copy(qp_sb, pqp)

            # ============ side chain (emitted at u==2) ================
            # qb[b] = q . bk + bq . bk, broadcast to [128, BS]. Only
            # consumed by the epilogue. (Mask processing is gone: the
            # host-built m01g already has the packed validity layout.)
            def _side_chain():
                # v[e'] = Wq.T @ bk, qb_raw = query @ v
                wq_sb = singles.tile([128, EC, E], f32)  # natural [p, h-chunk, e']
                for c in range(EC):
                    nc.scalar.dma_start(out=wq_sb[:, c, :], in_=wq_d[c * 128 : (c + 1) * 128, :])
                v_sb = singles.tile([128, EC], f32)
                for m in range(EC):
                    pv = ppre.tile([128, 1], f32, tag="pre")
                    for c in range(EC):
                        nc.tensor.matmul(pv, wq_sb[:, c, m * 128 : (m + 1) * 128],
                                         bkT[:, c : c + 1],
                                         start=(c == 0), stop=(c == EC - 1))
                    nc.vector.tensor_copy(v_sb[:, m : m + 1], pv)
                qb_sb = singles.tile([BS, 1], f32)
                pqbv = ppre.tile([BS, 1], f32, tag="pre")
                for m in range(EC):
                    nc.tensor.matmul(pqbv, qT_sb[:, m, :], v_sb[:, m : m + 1],
                                     start=(m == 0), stop=(m == EC - 1))
                nc.vector.tensor_copy(qb_sb, pqbv)
                # dot(bq, bk)
                trash4 = singles.tile([128, EC], f32)
                dotp = singles.tile([128, 1], f32)
                nc.vector.tensor_mul(trash4, bqT, bkT)
                nc.vector.tensor_reduce(dotp, trash4, axis=mybir.AxisListType.X, op=Alu.add)
                pdot = ppre.tile([1, 1], f32, tag="pre")
                nc.tensor.matmul(pdot, dotp, ones_col, start=True, stop=True)
                dot_sb = singles.tile([1, 1], f32)
                nc.vector.tensor_copy(dot_sb, pdot)
                pqbrow = ppre.tile([1, BS], f32, tag="pre")
                nc.tensor.transpose(pqbrow, qb_sb, ident[0:BS, 0:BS])
                qbrow_sb = singles.tile([1, BS], f32)
                nc.scalar.activation(qbrow_sb, pqbrow, Act.Identity,
                                     bias=dot_sb[0:1, 0:1], scale=1.0)
                pqbb = ppre.tile([128, BS], f32, tag="pre")
                nc.tensor.matmul(pqbb, ones_row, qbrow_sb, start=True, stop=True)
                qbb = singles.tile([128, BS], f32)
                nc.vector.tensor_copy(qbb, pqbb)
                st["qbb"] = qbb

            # ============ epilogue (two halves of 8 batches) ==========
            e2_sb = singles.tile([128, BS, KG], f32)
            a_sb = singles.tile([128, BS, KG], f32)
            part_sb = singles.tile([128, BS], f32)
            alpha_flat = alpha_d.rearrange("b (k p) -> (b k) p", p=128)
            HW = (BS // 2) * KG  # transpose width per half

            def _epi_half(half):
                b0, b1 = half * (BS // 2), (half + 1) * (BS // 2)
                eng_ts = nc.vector
                if skip_qb:
                    # one cheap DVE copy: scores reads stay confined to the
                    # VectorE program order, so later units' accum writes
                    # don't pick up cross-engine WAR stalls against the
                    # ScalarE tanh below
                    scores2 = singles.tile([128, BS // 2, KG], f32, tag=f"sc2_{half}")
                    nc.vector.tensor_copy(scores2, scores[:, b0:b1, :])
                    tanh_in = scores2
                else:
                    qbb = st["qbb"]
                    scores2 = singles.tile([128, BS // 2, KG], f32, tag=f"sc2_{half}")
                    for b in range(b0, b1):
                        eng_ts.tensor_scalar(
                            out=scores2[:, b - b0, :], in0=scores[:, b, :],
                            scalar1=qbb[:, b : b + 1], scalar2=None, op0=Alu.add,
                        )
                    tanh_in = scores2
                t_sb = singles.tile([128, BS // 2, KG], f32, tag=f"t_{half}")
                nc.scalar.activation(t_sb, tanh_in, Act.Tanh)
                nc.scalar.activation(e2_sb[:, b0:b1, :], t_sb, Act.Exp, scale=C_CLIP)
                nc.vector.tensor_mul(e2_sb[:, b0:b1, :], e2_sb[:, b0:b1, :],
                                     m01g_sb[:, b0:b1, :])
                # one 3D reduce over the innermost (chunk) dim replaces 8
                # serial per-batch reduces on the critical tail
                nc.vector.tensor_reduce(
                    part_sb[:, b0:b1], e2_sb[:, b0:b1, :],
                    axis=mybir.AxisListType.X, op=Alu.add,
                )
                pden = pepi.tile([1, BS // 2], f32, tag="epi")
                nc.tensor.matmul(pden, ones_col, part_sb[:, b0:b1], start=True, stop=True)
                recip_sb = singles.tile([1, BS // 2], f32, tag=f"rc_{half}")
                nc.vector.reciprocal(recip_sb, pden)
                prb = pepi.tile([128, BS // 2], f32, tag="epi")
                nc.tensor.matmul(prb, ones_row, recip_sb, start=True, stop=True)
                rb_sb = singles.tile([128, BS // 2], f32, tag=f"rb_{half}")
                nc.scalar.copy(rb_sb, prb)
                # one tensor_tensor with rb broadcast along the chunk dim
                # (stride-0 AP) replaces 8 serial per-batch muls
                rbb = bass.AP(
                    tensor=rb_sb.tensor, offset=rb_sb.offset,
                    ap=[rb_sb.ap[0], rb_sb.ap[1], [0, KG]],
                )
                eng_ts.tensor_mul(a_sb[:, b0:b1, :], e2_sb[:, b0:b1, :], rbb)
                pat = pepi.tile([128, 128], f32, tag="epi")
                nc.tensor.transpose(pat[0:HW, 0:128], a_sb[:, b0:b1, :], ident)
                at_sb = singles.tile([128, 128], f32, tag=f"at_{half}")
                nc.scalar.copy(at_sb[0:HW, 0:128], pat[0:HW, 0:128])
                nc.sync.dma_start(out=alpha_flat[half * HW : (half + 1) * HW, :],
                                  in_=at_sb[0:HW, 0:128])

            st = {}
            _emit_side_chain = _side_chain
            _epilogue_half = _epi_half

            # ================= main pipeline ==========================
            # Full-batch units: KG-chunk DMA -> KG fused
            # scalar_tensor_tensor ops on VectorE, each multiplying a
            # [128, 512] packed s-chunk by the SBUF-resident qp broadcast
            # with accum_out reducing over e in the same pass. out is a
            # stride-0 dummy column (the product is never stored).
            scores = singles.tile([128, BS, KG], f32)
            for u in range(n_units):
                b = u
                # broadcast qp[b] across 128 partitions with a single
                # 16-row matmul: lhsT = ident column b replicated along
                # the free dim (stride 0) selects row b of qp_sb into
                # every output partition
                sel = bass.AP(
                    tensor=ident.tensor,
                    offset=ident[0:BS, b : b + 1].offset,
                    ap=[[ident.ap[0][0], BS], [0, 128]],
                )
                pb_ps = pqpb.tile([128, E], f32, tag="qpb")
                nc.tensor.matmul(pb_ps, sel, qp_sb[:, :], start=True, stop=True)
                # stage the broadcast in SBUF: DVE pays 58-cycle access
                # latency per op instead of PSUM's 120
                pb_cur = pbsp.tile([128, E], f32, tag="pbs")
                nc.scalar.copy(pb_cur, pb_ps)
                tgt = tgtp.tile([128, KG, E], f32, tag="tgt")
                nc.sync.dma_start(out=tgt, in_=target_units[b])
                for k in range(KG):
                    tr = trashp.tile([128, 1], f32, tag="trash")
                    nc.vector.scalar_tensor_tensor(
                        out=tr.broadcast_to(tgt[:, k, :].shape),
                        in0=tgt[:, k, :], scalar=1.0, in1=pb_cur,
                        op0=Alu.mult, op1=Alu.mult,
                        accum_out=scores[:, b, k : k + 1],
                    )
                if u == 2 and not skip_qb:
                    _emit_side_chain()
                if u == 11:
                    _epilogue_half(0)

            _epilogue_half(1)

    _legalize_sync_waits(nc)
    return nc


_NC_CACHE = None
_NC_KEY = None


def _pack_inputs(query, target, mask, Wq, bq, Wk, bk):
    """Host-side restage: per-row unmasked indices, packed target rows,
    validity mask in the device's [p, b, k] layout."""
    idxs = [np.flatnonzero(mask[r] == 0) for r in range(B)]
    nmax = max((len(ix) for ix in idxs), default=1)
    nidx = max(128, ((nmax + 127) // 128) * 128)
    kg = nidx // 128
    packed = np.zeros((B, nidx, E), dtype=np.float32)
    m01g = np.zeros((128, B, kg), dtype=np.float32)
    for r in range(B):
        n = len(idxs[r])
        packed[r, :n] = target[r, idxs[r]]
        flat = np.zeros(nidx, dtype=np.float32)
        flat[:n] = 1.0
        m01g[:, r, :] = flat.reshape(kg, 128).T
    return idxs, nidx, packed, m01g


def kernel(query, target, mask, Wq, bq, Wk, bk):
    global _NC_CACHE, _NC_KEY
    _install_axon_profile_shim()
    from concourse.bass_utils import run_bass_kernel_spmd

    query = np.ascontiguousarray(np.asarray(query, dtype=np.float32))
    target = np.ascontiguousarray(np.asarray(target, dtype=np.float32))
    mask = np.ascontiguousarray(np.asarray(mask, dtype=np.int32))
    Wq = np.ascontiguousarray(np.asarray(Wq, dtype=np.float32))
    bq = np.ascontiguousarray(np.asarray(bq, dtype=np.float32))
    Wk = np.ascontiguousarray(np.asarray(Wk, dtype=np.float32))
    bk = np.ascontiguousarray(np.asarray(bk, dtype=np.float32))

    in_maps, idxs, nidx = _make_in_maps_impl(query, target, mask, Wq, bq, Wk, bk)

    skip_qb = bool(np.all(bk == 0.0))
    if _NC_CACHE is None or _NC_KEY != (nidx, skip_qb):
        _NC_CACHE = build_kernel(nidx, skip_qb)
        _NC_KEY = (nidx, skip_qb)
    nc = _NC_CACHE

    res = run_bass_kernel_spmd(nc, in_maps, list(range(NCORES)))
    out = np.zeros((B, S), dtype=np.float32)
    for i in range(NCORES):
        ag = res.results[i]["alpha"]
        for b in range(BS):
            r = i * BS + b
            n = len(idxs[r])
            out[r, idxs[r]] = ag[b, :n]
    return out


def _make_in_maps_impl(query, target, mask, Wq, bq, Wk, bk):
    idxs, nidx, packed, m01g = _pack_inputs(query, target, mask, Wq, bq, Wk, bk)
    WqT = np.ascontiguousarray(Wq.T)
    bqT = np.ascontiguousarray(bq.reshape(EC, 128).T)
    bkT = np.ascontiguousarray(bk.reshape(EC, 128).T)
    in_maps = []
    for i in range(NCORES):
        sl = slice(i * BS, (i + 1) * BS)
        in_maps.append(
            {
                "queryT": np.ascontiguousarray(query[sl].T),
                "target": np.ascontiguousarray(packed[sl]),
                "m01g": np.ascontiguousarray(m01g[:, sl, :]),
                "Wq": Wq,
                "WqT": WqT,
                "bqT": bqT,
                "Wk": Wk,
                "bkT": bkT,
            }
        )
    return in_maps, idxs, nidx


def make_in_maps(query, target, mask, Wq, bq, Wk, bk):
    in_maps, _, _ = _make_in_maps_impl(
        np.asarray(query, dtype=np.float32),
        np.asarray(target, dtype=np.float32),
        np.asarray(mask, dtype=np.int32),
        np.asarray(Wq, dtype=np.float32),
        np.asarray(bq, dtype=np.float32),
        np.asarray(Wk, dtype=np.float32),
        np.asarray(bk, dtype=np.float32),
    )
    return in_maps
